# revision 61
# baseline (speedup 1.0000x reference)
"""Trainium2 Bass kernel for GQA attention (B=2, L=2048, D=3072, H=24, KV=8,
HD=128, causal, half-split RoPE).

Sharding: TP=4 over heads x DP=2 over batch on 8 NeuronCores.
Core c = 4*b + s handles batch b with q-heads 6s..6s+5 and kv-heads 2s,2s+1.
Each core computes a partial o_proj output [L, D]; the host sums the 4 TP
partials per batch (the "all-reduce after o_proj" done on host at gather time).

v3: fp16 pipeline end-to-end; one 8-bank PSUM pool with per-tag buffers;
quarter-0 projections run d-outer across all 8 banks so PE paces the startup
DMA stream; engine queues are in-order, so next-quarter projections / V and
previous-quarter o_proj matmuls are sprinkled between attention chunks at
emission time to fill the exp-paced PE bubbles.

Per-core device computation (all matmuls fp16 with fp32 PSUM accumulation):
  xT[D,L] (host-pretransposed, fp16)
  Q^T = Wq_s^T x^T  (per head [128,L]) -> RoPE -> qT
  K^T likewise per kv head -> RoPE
  V   = x Wv_s   natural layout [L, 256]
  per head, per 512-wide q-block: S^T[k,q] chunks via PE, exp on ScalarE
  (scale folded into exp), causal mask on diagonal chunks, AV on PE,
  denominators accumulated on VectorE in fp16 (4x mode) + one ones-matmul,
  normalize into O^T fp16, then o_proj partial = O^T.T @ Wo_s -> [L, D]
  fp16 (host sums partials in fp32).
"""

import numpy as np

import concourse.mybir as mybir
import concourse.tile as tile
from concourse import bacc
from concourse.bass_utils import run_bass_kernel_spmd

F16NP = np.float16

B, L, D = 2, 2048, 3072
H, KV, HD = 24, 8, 128
GROUP = H // KV          # 3
THETA = 500000.0
SCALE = HD ** -0.5
N_CORES = 8
TP = 4                   # tensor-parallel over heads
NQH = H // TP            # 6 q heads per core
NKH = KV // TP           # 2 kv heads per core
QCOLS = NQH * HD         # 768
KCOLS = NKH * HD         # 256
ND = D // 128            # 24 contraction chunks
NLT = L // 128           # 16 l-tiles
NB = L // 512            # 4 q-blocks
NMI = NQH + NKH          # 8 projection column tiles
F16 = mybir.dt.float16
F32 = mybir.dt.float32


def _ls(i, w=512):
    return slice(i * w, (i + 1) * w)


def _rope_tables():
    half = HD // 2
    inv_freq = 1.0 / (THETA ** (np.arange(half, dtype=np.float64) / half))
    ang = np.arange(L, dtype=np.float64)[:, None] * inv_freq[None, :]  # [L, 64]
    cosT = np.cos(ang).T.astype(np.float32)   # [64, L]
    sinT = np.sin(ang).T.astype(np.float32)
    cosF = np.concatenate([cosT, cosT], 0)    # [128, L]
    sinF = np.concatenate([-sinT, sinT], 0)   # rows 0:64 get -sin
    return cosF.astype(F16NP), sinF.astype(F16NP)


def _mask_tiles():
    # Shifted-window causal mask base: for diagonal chunk offset m the mask
    # is mask[r, c] = (c >= 128*m + r); all four m-tiles are 128-shifted
    # windows of base[r, u] = (u >= r + 384), tile m = base[:, 384-128m:][:512]
    r = np.arange(128)[:, None]
    u = np.arange(896)[None, :]
    return (u >= r + 384).astype(F16NP)  # [128, 896]


def _emit(nc):
    xT = nc.dram_tensor("xT", [D, L], F16, kind="ExternalInput")
    wqk = nc.dram_tensor("wqk", [D, QCOLS + KCOLS], F16, kind="ExternalInput")
    wv = nc.dram_tensor("wv", [D, KCOLS], F16, kind="ExternalInput")
    wo = nc.dram_tensor("wo", [QCOLS, D], F16, kind="ExternalInput")
    out = nc.dram_tensor("out", [L, D], F16, kind="ExternalOutput")

    cosF, sinF = _rope_tables()
    cosc = nc.inline_tensor(np.ascontiguousarray(cosF), name="cosc")
    sinc = nc.inline_tensor(np.ascontiguousarray(sinF), name="sinc")
    maskc = nc.inline_tensor(np.ascontiguousarray(_mask_tiles()), name="maskc")

    Exp = mybir.ActivationFunctionType.Exp
    LQ = 512
    PTAGS = ["psqk", "psqk", "sc", "sc", "po", "po", "pp", "pp"]

    with tile.TileContext(nc) as tc:
        with (
            tc.tile_pool(name="persist", bufs=1) as P,
            tc.tile_pool(name="xt", bufs=2) as XT,
            tc.tile_pool(name="wres", bufs=1) as WR,
            tc.tile_pool(name="qtq", bufs=2) as QTQ,
            tc.tile_pool(name="oTq", bufs=2) as OTQ,
            tc.tile_pool(name="ropet", bufs=1) as RT,
            tc.tile_pool(name="p2", bufs=4) as P2,
            tc.tile_pool(name="stage", bufs=2) as SG,
            # One PSUM pool, 8 banks via per-tag bufs:
            #   psqk x2 (QK proj + V proj), sc x2 (scores + denom),
            #   po x2 (AV accum), pp x2 (o_proj).
            tc.tile_pool(name="ps", bufs=2, space="PSUM") as PS,
        ):
            ones_sb = P.tile([128, 128], F16, tag="ones")
            nc.vector.memset(ones_sb, 1.0)
            # warm the ACT Exp table + DVE reciprocal path during the
            # startup DMA window (the first real use would otherwise pay
            # the table load on the critical path; unmodeled in sim)
            warm = P.tile([128, 4], F32, tag="warm")
            nc.scalar.activation(warm[:, 0:1], ones_sb[:, 0:1], Exp,
                                 scale=1.0)
            nc.vector.reciprocal(warm[:, 1:2], warm[:, 0:1])
            # warm the PE clock (HAM / p-state ramp) with dummy matmuls on
            # the ones tile while the first wqk/xt chunks stream in, so the
            # real projection chains start at full clock (~3us ramp)
            warm_ps = PS.tile([128, 128], F32, tag="psqk", name="warmps")
            for i in range(24):
                nc.tensor.matmul(
                    warm_ps,
                    lhsT=ones_sb,
                    rhs=ones_sb,
                    start=(i == 0),
                    stop=(i == 23),
                    skip_group_check=True,
                )
            kT_sb = [
                P.tile([128, L], F16, tag=f"kT{i}", name=f"kT{i}")
                for i in range(NKH)
            ]
            v_sb = P.tile([128, NLT, KCOLS], F16, tag="vsb")

            wqk_sb = WR.tile([128, ND, QCOLS + KCOLS], F16, tag="wqksb")
            wqk_r = wqk.ap().rearrange("(dc p) n -> p dc n", p=128)
            wv_sb = WR.tile([128, ND, KCOLS], F16, tag="wvsb")
            wv_r = wv.ap().rearrange("(dc p) n -> p dc n", p=128)
            mask_sb = WR.tile([128, 896], F16, tag="mask")
            wo_sb = WR.tile([128, NQH, D], F16, tag="wosb")
            wo_r = wo.ap().rearrange("(c p) n -> p c n", p=128)
            xT_r = xT.ap().rearrange("(dc p) l -> p dc l", p=128)
            out_r = out.ap().rearrange(
                "(lt p) (et n) -> p lt et n", p=128, n=512
            )

            xt_tiles = {}
            qTq_tiles = {}
            oTq_tiles = {}
            cs_tiles = {}

            def load_cs(qt, eng):
                """Per-quarter cos/sin rope table slices."""
                hs = qt * LQ
                cosq = RT.tile([128, LQ], F16, tag="cosq", bufs=2, name="cosq")
                eng.dma_start(out=cosq, in_=cosc.ap()[:, hs:hs + LQ])
                sinq = RT.tile([128, LQ], F16, tag="sinq", bufs=2, name="sinq")
                eng.dma_start(out=sinq, in_=sinc.ap()[:, hs:hs + LQ])
                cs_tiles[qt] = (cosq, sinq)

            def load_xt(qt):
                # prefetch for qt>=1: not pacing-critical, so coarse 6-chunk
                # groups (fewer DMA instructions on the rings). Quarter 1's
                # prefetch rides the gated scalar queue: on sync/gpsimd it
                # would start as soon as a ring drains its startup-stream
                # share and steal DMA bandwidth from the stream's tail.
                xt_sb = XT.tile([128, ND, LQ], F16, tag="xt", name="xt_sb")
                xt_tiles[qt] = xt_sb
                hs = qt * LQ
                for g in range(ND // 6):
                    eng = (nc.scalar if qt == 1
                           else (nc.sync, nc.gpsimd)[g % 2])
                    eng.dma_start(
                        out=xt_sb[:, 6 * g:6 * g + 6, :],
                        in_=xT_r[:, 6 * g:6 * g + 6, hs:hs + LQ],
                    )
                load_cs(qt, nc.scalar)
                if qt == 1:
                    for c in range(NQH):
                        nc.scalar.dma_start(
                            out=wo_sb[:, c, :], in_=wo_r[:, c, :])

            def emit_rope(qt, mi, ps):
                """Drain psum chain mi -> rope -> qTq / kT."""
                hs = qt * LQ
                cosq, sinq = cs_tiles[qt]
                qkb = RT.tile([128, 512], F16, tag="qkb", name="qkb")
                nc.vector.tensor_copy(qkb, ps)
                rot = RT.tile([128, 512], F16, tag="rot", name="rot")
                nc.vector.tensor_copy(out=rot[0:64, :], in_=qkb[64:128, :])
                nc.vector.tensor_copy(out=rot[64:128, :], in_=qkb[0:64, :])
                t1 = RT.tile([128, 512], F16, tag="t1", name="t1")
                nc.vector.tensor_mul(t1, qkb, cosq)
                nc.vector.tensor_mul(rot, rot, sinq)
                dst = (qTq_tiles[qt][:, mi, :] if mi < NQH
                       else kT_sb[mi - NQH][:, hs:hs + LQ])
                nc.vector.tensor_add(dst, t1, rot)

            def gen_v(qt, step=2):
                """Generator: V projection chains for quarter qt."""
                xt_sb = xt_tiles[qt]
                for lt in range(LQ // 128):
                    glt = qt * (LQ // 128) + lt
                    pv = PS.tile([128, 512], F32, tag="psqk", name="pv")
                    for d in range(ND):
                        nc.tensor.matmul(
                            pv[:, 0:KCOLS],
                            lhsT=xt_sb[:, d, lt * 128:(lt + 1) * 128],
                            rhs=wv_sb[:, d, :],
                            start=(d == 0),
                            stop=(d == ND - 1),
                        )
                        if d % step == step - 1:
                            yield
                    nc.vector.tensor_copy(v_sb[:, glt, :], pv[:, 0:KCOLS])
                    yield

            def gen_proj(qt, with_v=True):
                """Generator: projection chains (kv heads first, so the
                flush-tail rope drains are late q heads that attention
                doesn't need immediately) + V for quarter qt, a couple of
                matmuls per yield. Quarter 0 is emitted eagerly instead."""
                qTq_tiles[qt] = QTQ.tile(
                    [128, NQH, LQ], F16, tag="qTq", name="qTq")
                xt_sb = xt_tiles[qt]
                for mi in (NQH, NQH + 1, *range(NQH)):
                    ps = PS.tile([128, 512], F32, tag="psqk", name="psqk")
                    for d in range(ND):
                        nc.tensor.matmul(
                            ps,
                            lhsT=wqk_sb[:, d, mi * 128:(mi + 1) * 128],
                            rhs=xt_sb[:, d, :],
                            start=(d == 0),
                            stop=(d == ND - 1),
                        )
                        yield
                    emit_rope(qt, mi, ps)
                if with_v:
                    yield from gen_v(qt)

            def gen_oproj(qt):
                """Generator: o_proj for quarter qt, ~one chain-step/yield.
                Output staged per half-l-tile (3 e-slices) so each store DMA
                writes 3KB-contiguous partition lines (3x fewer descriptors
                than per-e stores)."""
                oTq = oTq_tiles[qt]
                for lt in range(LQ // 128):
                    glt = qt * (LQ // 128) + lt
                    for eh in range(2):
                        st3 = SG.tile([128, 3, 512], F16, tag="st",
                                      name="st3")
                        for e3 in range(3):
                            e = eh * 3 + e3
                            pp = PS.tile([128, 512], F32, tag="pp", name="pp")
                            for c in range(NQH):
                                nc.tensor.matmul(
                                    pp,
                                    lhsT=oTq[:, c, lt * 128:(lt + 1) * 128],
                                    rhs=wo_sb[:, c, _ls(e)],
                                    start=(c == 0),
                                    stop=(c == NQH - 1),
                                )
                                yield
                            # qt3's o_proj runs after all exps: ACT is idle,
                            # while DVE still has the last head's backlog
                            if qt == NB - 1 or e % 2 == 1:
                                nc.scalar.copy(st3[:, e3, :], pp)
                            else:
                                nc.vector.tensor_copy(st3[:, e3, :], pp)
                            last = (qt == NB - 1 and lt == LQ // 128 - 1
                                    and eh == 1)
                            if last:
                                # final half-l-tile: store per-e so the last
                                # transfer on the critical tail is 3x smaller
                                nc.sync.dma_start(
                                    out=out_r[:, glt, e, :],
                                    in_=st3[:, e3, :],
                                )
                        if not last:
                            # keep out-store DMA issue off the scalar ring:
                            # ACT's in-order SEQ would delay exps behind the
                            # descriptor generation
                            oeng = (nc.gpsimd, nc.sync)[(glt * 2 + eh) % 2]
                            oeng.dma_start(
                                out=out_r[:, glt, 3 * eh:3 * eh + 3, :],
                                in_=st3,
                            )

            # Two filler queues: proj fillers must complete before the next
            # quarter's attention (flushed at quarter end); o_proj fillers
            # can linger to feed later quarters' bubbles.
            fill_proj = []
            fill_oproj = []

            def take(n):
                """Emit up to n filler steps (each ~1-2 ready PE matmuls)."""
                while n > 0:
                    q = fill_proj if fill_proj else fill_oproj
                    if not q:
                        return
                    try:
                        next(q[0])
                        n -= 1
                    except StopIteration:
                        q.pop(0)

            def flush_proj():
                while fill_proj:
                    try:
                        next(fill_proj[0])
                    except StopIteration:
                        fill_proj.pop(0)

            def flush_all():
                flush_proj()
                while fill_oproj:
                    try:
                        next(fill_oproj[0])
                    except StopIteration:
                        fill_oproj.pop(0)

            # ---- startup: quarter-0 projections d-outer across 8 banks,
            # with (wqk[d], xt[d]) DMA pairs interleaved so chunk arrivals
            # pace the 8 accumulation chains ----
            xt0 = XT.tile([128, ND, LQ], F16, tag="xt", name="xt_sb0")
            xt_tiles[0] = xt0
            qTq_tiles[0] = QTQ.tile([128, NQH, LQ], F16, tag="qTq",
                                    name="qTq0")
            ps_mi = [
                PS.tile([128, 512], F32, tag=PTAGS[mi], name=f"ps{mi}")
                for mi in range(NMI)
            ]
            for d in range(ND):
                eng = (nc.sync, nc.gpsimd)[d % 2]
                eng.dma_start(out=wqk_sb[:, d, :], in_=wqk_r[:, d, :])
                xeng = (nc.gpsimd, nc.sync)[d % 2]
                xeng.dma_start(out=xt0[:, d, :], in_=xT_r[:, d, 0:LQ])
                for mi in range(NMI):
                    nc.tensor.matmul(
                        ps_mi[mi],
                        lhsT=wqk_sb[:, d, mi * 128:(mi + 1) * 128],
                        rhs=xt0[:, d, :],
                        start=(d == 0),
                        stop=(d == ND - 1),
                    )
                if d == 12:
                    # Secondary loads on the scalar ring. Emission position
                    # alone does NOT delay runtime issue (the scalar queue
                    # has no deps and would fire at t=0, contending with the
                    # startup stream for DMA bandwidth) — gate them on the
                    # stream's LAST chunk: the uncontended stream finishes
                    # ~12us before the PE drains it, and all secondary
                    # consumers (wv/cos/sin@44us, mask@48, wo@75) still
                    # clear their deadlines loading in that shadow.
                    nc.scalar.copy(warm[:, 2:3], wqk_sb[:, ND - 1, 0:1])
                    for g in range(0, ND, 4):
                        nc.scalar.dma_start(
                            out=wv_sb[:, g:g + 4, :], in_=wv_r[:, g:g + 4, :]
                        )
                    nc.scalar.dma_start(out=mask_sb, in_=maskc.ap())
                    load_cs(0, nc.scalar)
                    # wo is emitted later (after xt(1)) so the scalar queue
                    # orders post-stream transfers by consumer deadline
            # Interleave rope drains with V-projection chains: V chain lt
            # rotates onto the psqk banks, so ropes 0/1 go first, and the
            # kv-head ropes (6/7) land between V chains ahead of attention.
            def emit_v0(lt):
                pv = PS.tile([128, 512], F32, tag="psqk", name="pv0")
                for d in range(ND):
                    nc.tensor.matmul(
                        pv[:, 0:KCOLS],
                        lhsT=xt0[:, d, lt * 128:(lt + 1) * 128],
                        rhs=wv_sb[:, d, :],
                        start=(d == 0),
                        stop=(d == ND - 1),
                    )
                nc.vector.tensor_copy(v_sb[:, lt, :], pv[:, 0:KCOLS])

            emit_rope(0, 0, ps_mi[0])
            emit_rope(0, 1, ps_mi[1])
            emit_v0(0)
            emit_rope(0, NQH, ps_mi[NQH])
            emit_v0(1)
            emit_rope(0, NQH + 1, ps_mi[NQH + 1])
            emit_v0(2)
            emit_rope(0, 2, ps_mi[2])
            emit_v0(3)
            for mi in (3, 4, 5):
                emit_rope(0, mi, ps_mi[mi])

            # ---- quarter loop: attention(q) with sprinkled fillers ----
            for qt in range(NB):
                b = qt
                nch = 4 * (b + 1)
                oTq_tiles[qt] = OTQ.tile(
                    [128, NQH, LQ], F16, tag="oTq", name="oTq")
                if qt < NB - 1:
                    load_xt(qt + 1)
                    if qt > 0:
                        fill_proj.append(gen_proj(qt + 1))
                qTq = qTq_tiles[qt]
                oTq = oTq_tiles[qt]
                for h in range(NQH):
                    if qt == 0 and h == 3:
                        # xt(1) has landed by now; safe to sprinkle proj(1)
                        fill_proj.append(gen_proj(1))
                    kv = h // GROUP
                    po = PS.tile([128, 512], F32, tag="po", name="po")
                    acc = P2.tile([128, 512], F16, tag="acc", bufs=2,
                                  name="acc")
                    # Diagonal chunk m = j-4b computes only its valid column
                    # window [128m, 512): rows k=128j+r allow q >= 128m+r, so
                    # columns below 128m are fully masked. The shifted-base
                    # mask makes every diag window's mask slice start at 384.
                    # AV(j) is software-pipelined one chunk behind SC(j): the
                    # next chunk's SC covers exp(j)'s latency, so mid-head
                    # filler take() drops to 1 and the supply lasts into qt3.
                    pend_av = None
                    for j in range(nch):
                        m = j - 4 * b
                        c0 = 0 if m < 0 else 128 * m
                        w = 512 - c0
                        sc = PS.tile([128, 512], F32, tag="sc", name="sc")
                        nc.tensor.matmul(
                            sc[:, 0:w],
                            lhsT=kT_sb[kv][:, j * 128:(j + 1) * 128],
                            rhs=qTq[:, h, c0:512],
                            start=True,
                            stop=True,
                        )
                        take(2 if (j == 0 or m >= 0) else 1)
                        pt = P2.tile([128, w], F16, tag="pt", bufs=4,
                                     name="pt")
                        nc.scalar.activation(pt, sc[:, 0:w], Exp, scale=SCALE)
                        if m >= 0:
                            nc.vector.tensor_mul(
                                pt, pt, mask_sb[:, 384:384 + w]
                            )
                        # per-k partial denominators accumulate on DVE
                        # (fp16 SBUF-only: 2x mode)
                        if j == 0:
                            nc.vector.tensor_copy(acc, pt)
                        else:
                            nc.vector.tensor_add(
                                acc[:, c0:512], acc[:, c0:512], pt
                            )
                        if pend_av is not None:
                            pend_av()
                        pend_av = (lambda jj, ptc, cc0: lambda:
                                   nc.tensor.matmul(
                                       po[:, cc0:512],
                                       lhsT=v_sb[:, jj,
                                                 kv * 128:(kv + 1) * 128],
                                       rhs=ptc,
                                       start=(jj == 0),
                                       stop=(jj == nch - 1),
                                       skip_group_check=(cc0 > 0),
                                   ))(j, pt, c0)
                    pend_av()
                    take(2)
                    # partition-reduce the denominators on PE
                    psm = PS.tile([128, 512], F32, tag="sc", name="psm")
                    nc.tensor.matmul(
                        psm, lhsT=ones_sb, rhs=acc, start=True, stop=True
                    )
                    rc = P2.tile([128, 512], F32, tag="rc", bufs=1, name="rc")
                    nc.vector.reciprocal(rc, psm)
                    nc.vector.tensor_mul(oTq[:, h, :], po, rc)
                    take(2)
                flush_proj()
                fill_oproj.append(gen_oproj(qt))
                if qt == NB - 1:
                    flush_all()
    return nc


_NC_CACHE = {}


def build():
    key = "v3"
    if key not in _NC_CACHE:
        nc = bacc.Bacc(
            "TRN2", target_bir_lowering=False, debug=False, num_devices=N_CORES
        )
        _emit(nc)
        nc.compile()
        _NC_CACHE[key] = nc
    return _NC_CACHE[key]


def prep_in_maps(x, Wq, Wk, Wv, Wo):
    """Shard + cast + layout the full inputs into 8 per-core input maps."""
    x = np.asarray(x)
    Wq, Wk, Wv, Wo = (np.asarray(a) for a in (Wq, Wk, Wv, Wo))
    in_maps = []
    wqk_s = [
        np.ascontiguousarray(np.hstack([
            Wq[:, s * QCOLS:(s + 1) * QCOLS],
            Wk[:, s * KCOLS:(s + 1) * KCOLS],
        ])).astype(F16NP)
        for s in range(TP)
    ]
    wv_s = [np.ascontiguousarray(Wv[:, s * KCOLS:(s + 1) * KCOLS]).astype(F16NP)
            for s in range(TP)]
    wo_s = [np.ascontiguousarray(Wo[s * QCOLS:(s + 1) * QCOLS, :]).astype(F16NP)
            for s in range(TP)]
    xT_b = [np.ascontiguousarray(x[b].T).astype(F16NP) for b in range(B)]
    for core in range(N_CORES):
        b, s = divmod(core, TP)
        in_maps.append({
            "xT": xT_b[b],
            "wqk": wqk_s[s],
            "wv": wv_s[s],
            "wo": wo_s[s],
        })
    return in_maps


def kernel(x, Wq, Wk, Wv, Wo):
    nc = build()
    in_maps = prep_in_maps(x, Wq, Wk, Wv, Wo)
    res = run_bass_kernel_spmd(nc, in_maps, list(range(N_CORES)))
    out = np.zeros((B, L, D), np.float32)
    for core in range(N_CORES):
        b, _s = divmod(core, TP)
        out[b] += res.results[core]["out"].astype(np.float32)
    return out


# revision 62
# speedup vs baseline: 1.1161x; 1.1161x over previous
"""Trainium2 Bass kernel for GQA attention (B=2, L=2048, D=3072, H=24, KV=8,
HD=128, causal, half-split RoPE).

Sharding: TP=4 over heads x DP=2 over batch on 8 NeuronCores.
Core c = 4*b + s handles batch b with q-heads 6s..6s+5 and kv-heads 2s,2s+1.
Each core computes a partial o_proj output [L, D]; the host sums the 4 TP
partials per batch (the "all-reduce after o_proj" done on host at gather time).

v3: fp16 pipeline end-to-end; one 8-bank PSUM pool with per-tag buffers;
quarter-0 projections run d-outer across all 8 banks so PE paces the startup
DMA stream; engine queues are in-order, so next-quarter projections / V and
previous-quarter o_proj matmuls are sprinkled between attention chunks at
emission time to fill the exp-paced PE bubbles.

Per-core device computation (all matmuls fp16 with fp32 PSUM accumulation):
  xT[D,L] (host-pretransposed, fp16)
  Q^T = Wq_s^T x^T  (per head [128,L]) -> RoPE -> qT
  K^T likewise per kv head -> RoPE
  V   = x Wv_s   natural layout [L, 256]
  per head, per 512-wide q-block: S^T[k,q] chunks via PE, exp on ScalarE
  (scale folded into exp), causal mask on diagonal chunks, AV on PE,
  denominators accumulated on VectorE in fp16 (4x mode) + one ones-matmul,
  normalize into O^T fp16, then o_proj partial = O^T.T @ Wo_s -> [L, D]
  fp16 (host sums partials in fp32).
"""

import numpy as np

import concourse.mybir as mybir
import concourse.tile as tile
from concourse import bacc
from concourse.bass_utils import run_bass_kernel_spmd

F16NP = np.float16

B, L, D = 2, 2048, 3072
H, KV, HD = 24, 8, 128
GROUP = H // KV          # 3
THETA = 500000.0
SCALE = HD ** -0.5
N_CORES = 8
TP = 4                   # tensor-parallel over heads
NQH = H // TP            # 6 q heads per core
NKH = KV // TP           # 2 kv heads per core
QCOLS = NQH * HD         # 768
KCOLS = NKH * HD         # 256
ND = D // 128            # 24 contraction chunks
NLT = L // 128           # 16 l-tiles
NB = L // 512            # 4 q-blocks
NMI = NQH + NKH          # 8 projection column tiles
F16 = mybir.dt.float16
F32 = mybir.dt.float32


def _ls(i, w=512):
    return slice(i * w, (i + 1) * w)


def _rope_tables():
    half = HD // 2
    inv_freq = 1.0 / (THETA ** (np.arange(half, dtype=np.float64) / half))
    ang = np.arange(L, dtype=np.float64)[:, None] * inv_freq[None, :]  # [L, 64]
    cosT = np.cos(ang).T.astype(np.float32)   # [64, L]
    sinT = np.sin(ang).T.astype(np.float32)
    cosF = np.concatenate([cosT, cosT], 0)    # [128, L]
    sinF = np.concatenate([-sinT, sinT], 0)   # rows 0:64 get -sin
    return cosF.astype(F16NP), sinF.astype(F16NP)


def _mask_tiles():
    # Shifted-window causal mask base: for diagonal chunk offset m the mask
    # is mask[r, c] = (c >= 128*m + r); all four m-tiles are 128-shifted
    # windows of base[r, u] = (u >= r + 384), tile m = base[:, 384-128m:][:512]
    r = np.arange(128)[:, None]
    u = np.arange(896)[None, :]
    return (u >= r + 384).astype(F16NP)  # [128, 896]


def _emit(nc):
    xT = nc.dram_tensor("xT", [D, L], F16, kind="ExternalInput")
    wqk = nc.dram_tensor("wqk", [D, QCOLS + KCOLS], F16, kind="ExternalInput")
    wv = nc.dram_tensor("wv", [D, KCOLS], F16, kind="ExternalInput")
    wo = nc.dram_tensor("wo", [QCOLS, D], F16, kind="ExternalInput")
    out = nc.dram_tensor("out", [L, D], F16, kind="ExternalOutput")

    cosF, sinF = _rope_tables()
    cosc = nc.inline_tensor(np.ascontiguousarray(cosF), name="cosc")
    sinc = nc.inline_tensor(np.ascontiguousarray(sinF), name="sinc")
    maskc = nc.inline_tensor(np.ascontiguousarray(_mask_tiles()), name="maskc")

    Exp = mybir.ActivationFunctionType.Exp
    LQ = 512
    PTAGS = ["psqk", "psqk", "sc", "sc", "po", "po", "pp", "pp"]

    with tile.TileContext(nc) as tc:
        with (
            tc.tile_pool(name="persist", bufs=1) as P,
            tc.tile_pool(name="xt", bufs=2) as XT,
            tc.tile_pool(name="wres", bufs=1) as WR,
            tc.tile_pool(name="qtq", bufs=2) as QTQ,
            tc.tile_pool(name="oTq", bufs=2) as OTQ,
            tc.tile_pool(name="ropet", bufs=1) as RT,
            tc.tile_pool(name="p2", bufs=4) as P2,
            tc.tile_pool(name="stage", bufs=2) as SG,
            # One PSUM pool, 8 banks via per-tag bufs:
            #   psqk x2 (QK proj + V proj), sc x2 (scores + denom),
            #   po x2 (AV accum), pp x2 (o_proj).
            tc.tile_pool(name="ps", bufs=2, space="PSUM") as PS,
        ):
            ones_sb = P.tile([128, 128], F16, tag="ones")
            nc.vector.memset(ones_sb, 1.0)
            # warm the ACT Exp table + DVE reciprocal path during the
            # startup DMA window (the first real use would otherwise pay
            # the table load on the critical path; unmodeled in sim)
            warm = P.tile([128, 4], F32, tag="warm")
            nc.scalar.activation(warm[:, 0:1], ones_sb[:, 0:1], Exp,
                                 scale=1.0)
            nc.vector.reciprocal(warm[:, 1:2], warm[:, 0:1])
            # warm the PE clock (HAM / p-state ramp) with dummy matmuls on
            # the ones tile while the first wqk/xt chunks stream in, so the
            # real projection chains start at full clock (~3us ramp)
            warm_ps = PS.tile([128, 128], F32, tag="psqk", name="warmps")
            for i in range(24):
                nc.tensor.matmul(
                    warm_ps,
                    lhsT=ones_sb,
                    rhs=ones_sb,
                    start=(i == 0),
                    stop=(i == 23),
                    skip_group_check=True,
                )
            kT_sb = [
                P.tile([128, L], F16, tag=f"kT{i}", name=f"kT{i}")
                for i in range(NKH)
            ]
            v_sb = P.tile([128, NLT, KCOLS], F16, tag="vsb")

            wqk_sb = WR.tile([128, ND, QCOLS + KCOLS], F16, tag="wqksb")
            wqk_r = wqk.ap().rearrange("(dc p) n -> p dc n", p=128)
            wv_sb = WR.tile([128, ND, KCOLS], F16, tag="wvsb")
            wv_r = wv.ap().rearrange("(dc p) n -> p dc n", p=128)
            mask_sb = WR.tile([128, 896], F16, tag="mask")
            wo_sb = WR.tile([128, NQH, D], F16, tag="wosb")
            wo_r = wo.ap().rearrange("(c p) n -> p c n", p=128)
            xT_r = xT.ap().rearrange("(dc p) l -> p dc l", p=128)
            out_r = out.ap().rearrange(
                "(lt p) (et n) -> p lt et n", p=128, n=512
            )

            xt_tiles = {}
            qTq_tiles = {}
            oTq_tiles = {}
            cs_tiles = {}

            def load_cs(qt, eng):
                """Per-quarter cos/sin rope table slices."""
                hs = qt * LQ
                cosq = RT.tile([128, LQ], F16, tag="cosq", bufs=2, name="cosq")
                eng.dma_start(out=cosq, in_=cosc.ap()[:, hs:hs + LQ])
                sinq = RT.tile([128, LQ], F16, tag="sinq", bufs=2, name="sinq")
                eng.dma_start(out=sinq, in_=sinc.ap()[:, hs:hs + LQ])
                cs_tiles[qt] = (cosq, sinq)

            def load_xt(qt):
                # prefetch for qt>=1: not pacing-critical, so coarse 6-chunk
                # groups (fewer DMA instructions on the rings). Quarter 1's
                # prefetch rides the gated scalar queue: on sync/gpsimd it
                # would start as soon as a ring drains its startup-stream
                # share and steal DMA bandwidth from the stream's tail.
                xt_sb = XT.tile([128, ND, LQ], F16, tag="xt", name="xt_sb")
                xt_tiles[qt] = xt_sb
                hs = qt * LQ
                # ALL prefetches ride the gated scalar queue: on sync/gpsimd
                # they'd start the moment the ring drains its startup-stream
                # share and contend with the stream tail (xt(2)'s transfers
                # otherwise begin at ~24us). Deadline-ordered behind the
                # gate, every consumer still clears with margin.
                for g in range(ND // 6):
                    nc.scalar.dma_start(
                        out=xt_sb[:, 6 * g:6 * g + 6, :],
                        in_=xT_r[:, 6 * g:6 * g + 6, hs:hs + LQ],
                    )
                load_cs(qt, nc.scalar)
                if qt == 1:
                    for c in range(NQH):
                        nc.scalar.dma_start(
                            out=wo_sb[:, c, :], in_=wo_r[:, c, :])

            def emit_rope(qt, mi, ps):
                """Drain psum chain mi -> rope -> qTq / kT."""
                hs = qt * LQ
                cosq, sinq = cs_tiles[qt]
                qkb = RT.tile([128, 512], F16, tag="qkb", name="qkb")
                nc.vector.tensor_copy(qkb, ps)
                rot = RT.tile([128, 512], F16, tag="rot", name="rot")
                nc.vector.tensor_copy(out=rot[0:64, :], in_=qkb[64:128, :])
                nc.vector.tensor_copy(out=rot[64:128, :], in_=qkb[0:64, :])
                t1 = RT.tile([128, 512], F16, tag="t1", name="t1")
                nc.vector.tensor_mul(t1, qkb, cosq)
                nc.vector.tensor_mul(rot, rot, sinq)
                dst = (qTq_tiles[qt][:, mi, :] if mi < NQH
                       else kT_sb[mi - NQH][:, hs:hs + LQ])
                nc.vector.tensor_add(dst, t1, rot)

            def gen_v(qt, step=2):
                """Generator: V projection chains for quarter qt."""
                xt_sb = xt_tiles[qt]
                for lt in range(LQ // 128):
                    glt = qt * (LQ // 128) + lt
                    pv = PS.tile([128, 512], F32, tag="psqk", name="pv")
                    for d in range(ND):
                        nc.tensor.matmul(
                            pv[:, 0:KCOLS],
                            lhsT=xt_sb[:, d, lt * 128:(lt + 1) * 128],
                            rhs=wv_sb[:, d, :],
                            start=(d == 0),
                            stop=(d == ND - 1),
                        )
                        if d % step == step - 1:
                            yield
                    nc.vector.tensor_copy(v_sb[:, glt, :], pv[:, 0:KCOLS])
                    yield

            def gen_proj(qt, with_v=True):
                """Generator: projection chains (kv heads first, so the
                flush-tail rope drains are late q heads that attention
                doesn't need immediately) + V for quarter qt, a couple of
                matmuls per yield. Quarter 0 is emitted eagerly instead."""
                qTq_tiles[qt] = QTQ.tile(
                    [128, NQH, LQ], F16, tag="qTq", name="qTq")
                xt_sb = xt_tiles[qt]
                for mi in (NQH, NQH + 1, *range(NQH)):
                    ps = PS.tile([128, 512], F32, tag="psqk", name="psqk")
                    for d in range(ND):
                        nc.tensor.matmul(
                            ps,
                            lhsT=wqk_sb[:, d, mi * 128:(mi + 1) * 128],
                            rhs=xt_sb[:, d, :],
                            start=(d == 0),
                            stop=(d == ND - 1),
                        )
                        yield
                    emit_rope(qt, mi, ps)
                if with_v:
                    yield from gen_v(qt)

            def gen_oproj(qt):
                """Generator: o_proj for quarter qt, ~one chain-step/yield.
                Output staged per half-l-tile (3 e-slices) so each store DMA
                writes 3KB-contiguous partition lines (3x fewer descriptors
                than per-e stores)."""
                oTq = oTq_tiles[qt]
                for lt in range(LQ // 128):
                    glt = qt * (LQ // 128) + lt
                    for eh in range(2):
                        st3 = SG.tile([128, 3, 512], F16, tag="st",
                                      name="st3")
                        for e3 in range(3):
                            e = eh * 3 + e3
                            pp = PS.tile([128, 512], F32, tag="pp", name="pp")
                            for c in range(NQH):
                                nc.tensor.matmul(
                                    pp,
                                    lhsT=oTq[:, c, lt * 128:(lt + 1) * 128],
                                    rhs=wo_sb[:, c, _ls(e)],
                                    start=(c == 0),
                                    stop=(c == NQH - 1),
                                )
                                yield
                            # qt3's o_proj runs after all exps: ACT is idle,
                            # while DVE still has the last head's backlog
                            if qt == NB - 1 or e % 2 == 1:
                                nc.scalar.copy(st3[:, e3, :], pp)
                            else:
                                nc.vector.tensor_copy(st3[:, e3, :], pp)
                            last = (qt == NB - 1 and lt == LQ // 128 - 1
                                    and eh == 1)
                            if last:
                                # final half-l-tile: store per-e so the last
                                # transfer on the critical tail is 3x smaller
                                nc.sync.dma_start(
                                    out=out_r[:, glt, e, :],
                                    in_=st3[:, e3, :],
                                )
                        if not last:
                            # keep out-store DMA issue off the scalar ring:
                            # ACT's in-order SEQ would delay exps behind the
                            # descriptor generation
                            oeng = (nc.gpsimd, nc.sync)[(glt * 2 + eh) % 2]
                            oeng.dma_start(
                                out=out_r[:, glt, 3 * eh:3 * eh + 3, :],
                                in_=st3,
                            )

            # Two filler queues: proj fillers must complete before the next
            # quarter's attention (flushed at quarter end); o_proj fillers
            # can linger to feed later quarters' bubbles.
            fill_proj = []
            fill_oproj = []

            def take(n):
                """Emit up to n filler steps (each ~1-2 ready PE matmuls)."""
                while n > 0:
                    q = fill_proj if fill_proj else fill_oproj
                    if not q:
                        return
                    try:
                        next(q[0])
                        n -= 1
                    except StopIteration:
                        q.pop(0)

            def flush_proj():
                while fill_proj:
                    try:
                        next(fill_proj[0])
                    except StopIteration:
                        fill_proj.pop(0)

            def flush_all():
                flush_proj()
                while fill_oproj:
                    try:
                        next(fill_oproj[0])
                    except StopIteration:
                        fill_oproj.pop(0)

            # ---- startup: quarter-0 projections d-outer across 8 banks,
            # with (wqk[d], xt[d]) DMA pairs interleaved so chunk arrivals
            # pace the 8 accumulation chains ----
            xt0 = XT.tile([128, ND, LQ], F16, tag="xt", name="xt_sb0")
            xt_tiles[0] = xt0
            qTq_tiles[0] = QTQ.tile([128, NQH, LQ], F16, tag="qTq",
                                    name="qTq0")
            ps_mi = [
                PS.tile([128, 512], F32, tag=PTAGS[mi], name=f"ps{mi}")
                for mi in range(NMI)
            ]
            for d in range(ND):
                eng = (nc.sync, nc.gpsimd)[d % 2]
                eng.dma_start(out=wqk_sb[:, d, :], in_=wqk_r[:, d, :])
                xeng = (nc.gpsimd, nc.sync)[d % 2]
                xeng.dma_start(out=xt0[:, d, :], in_=xT_r[:, d, 0:LQ])
                for mi in range(NMI):
                    nc.tensor.matmul(
                        ps_mi[mi],
                        lhsT=wqk_sb[:, d, mi * 128:(mi + 1) * 128],
                        rhs=xt0[:, d, :],
                        start=(d == 0),
                        stop=(d == ND - 1),
                    )
                if d == 12:
                    # Secondary loads on the scalar ring. Emission position
                    # alone does NOT delay runtime issue (the scalar queue
                    # has no deps and would fire at t=0, contending with the
                    # startup stream for DMA bandwidth) — gate them on the
                    # stream's LAST chunk: the uncontended stream finishes
                    # ~12us before the PE drains it, and all secondary
                    # consumers (wv/cos/sin@44us, mask@48, wo@75) still
                    # clear their deadlines loading in that shadow.
                    nc.scalar.copy(warm[:, 2:3], wqk_sb[:, ND - 1, 0:1])
                    for g in range(0, ND, 4):
                        nc.scalar.dma_start(
                            out=wv_sb[:, g:g + 4, :], in_=wv_r[:, g:g + 4, :]
                        )
                    nc.scalar.dma_start(out=mask_sb, in_=maskc.ap())
                    load_cs(0, nc.scalar)
                    # wo is emitted later (after xt(1)) so the scalar queue
                    # orders post-stream transfers by consumer deadline
            # Interleave rope drains with V-projection chains: V chain lt
            # rotates onto the psqk banks, so ropes 0/1 go first, and the
            # kv-head ropes (6/7) land between V chains ahead of attention.
            def emit_v0(lt):
                pv = PS.tile([128, 512], F32, tag="psqk", name="pv0")
                for d in range(ND):
                    nc.tensor.matmul(
                        pv[:, 0:KCOLS],
                        lhsT=xt0[:, d, lt * 128:(lt + 1) * 128],
                        rhs=wv_sb[:, d, :],
                        start=(d == 0),
                        stop=(d == ND - 1),
                    )
                nc.vector.tensor_copy(v_sb[:, lt, :], pv[:, 0:KCOLS])

            emit_rope(0, 0, ps_mi[0])
            emit_rope(0, 1, ps_mi[1])
            emit_v0(0)
            emit_rope(0, NQH, ps_mi[NQH])
            emit_v0(1)
            emit_rope(0, NQH + 1, ps_mi[NQH + 1])
            emit_v0(2)
            emit_rope(0, 2, ps_mi[2])
            emit_v0(3)
            for mi in (3, 4, 5):
                emit_rope(0, mi, ps_mi[mi])

            # ---- quarter loop: attention(q) with sprinkled fillers ----
            for qt in range(NB):
                b = qt
                nch = 4 * (b + 1)
                oTq_tiles[qt] = OTQ.tile(
                    [128, NQH, LQ], F16, tag="oTq", name="oTq")
                if qt < NB - 1:
                    load_xt(qt + 1)
                    if qt > 0:
                        fill_proj.append(gen_proj(qt + 1))
                qTq = qTq_tiles[qt]
                oTq = oTq_tiles[qt]
                for h in range(NQH):
                    if qt == 0 and h == 3:
                        # xt(1) has landed by now; safe to sprinkle proj(1)
                        fill_proj.append(gen_proj(1))
                    kv = h // GROUP
                    po = PS.tile([128, 512], F32, tag="po", name="po")
                    acc = P2.tile([128, 512], F16, tag="acc", bufs=2,
                                  name="acc")
                    # Diagonal chunk m = j-4b computes only its valid column
                    # window [128m, 512): rows k=128j+r allow q >= 128m+r, so
                    # columns below 128m are fully masked. The shifted-base
                    # mask makes every diag window's mask slice start at 384.
                    # AV(j) is software-pipelined one chunk behind SC(j): the
                    # next chunk's SC covers exp(j)'s latency, so mid-head
                    # filler take() drops to 1 and the supply lasts into qt3.
                    pend_av = None
                    for j in range(nch):
                        m = j - 4 * b
                        c0 = 0 if m < 0 else 128 * m
                        w = 512 - c0
                        sc = PS.tile([128, 512], F32, tag="sc", name="sc")
                        nc.tensor.matmul(
                            sc[:, 0:w],
                            lhsT=kT_sb[kv][:, j * 128:(j + 1) * 128],
                            rhs=qTq[:, h, c0:512],
                            start=True,
                            stop=True,
                        )
                        take(2 if (j == 0 or m >= 0) else 1)
                        pt = P2.tile([128, w], F16, tag="pt", bufs=4,
                                     name="pt")
                        nc.scalar.activation(pt, sc[:, 0:w], Exp, scale=SCALE)
                        if m >= 0:
                            nc.vector.tensor_mul(
                                pt, pt, mask_sb[:, 384:384 + w]
                            )
                        # per-k partial denominators accumulate on DVE
                        # (fp16 SBUF-only: 2x mode)
                        if j == 0:
                            nc.vector.tensor_copy(acc, pt)
                        else:
                            nc.vector.tensor_add(
                                acc[:, c0:512], acc[:, c0:512], pt
                            )
                        if pend_av is not None:
                            pend_av()
                        pend_av = (lambda jj, ptc, cc0: lambda:
                                   nc.tensor.matmul(
                                       po[:, cc0:512],
                                       lhsT=v_sb[:, jj,
                                                 kv * 128:(kv + 1) * 128],
                                       rhs=ptc,
                                       start=(jj == 0),
                                       stop=(jj == nch - 1),
                                       skip_group_check=(cc0 > 0),
                                   ))(j, pt, c0)
                    pend_av()
                    take(2)
                    # partition-reduce the denominators on PE
                    psm = PS.tile([128, 512], F32, tag="sc", name="psm")
                    nc.tensor.matmul(
                        psm, lhsT=ones_sb, rhs=acc, start=True, stop=True
                    )
                    rc = P2.tile([128, 512], F32, tag="rc", bufs=1, name="rc")
                    nc.vector.reciprocal(rc, psm)
                    nc.vector.tensor_mul(oTq[:, h, :], po, rc)
                    take(2)
                flush_proj()
                fill_oproj.append(gen_oproj(qt))
                if qt == NB - 1:
                    flush_all()
    return nc


_NC_CACHE = {}


def build():
    key = "v3"
    if key not in _NC_CACHE:
        nc = bacc.Bacc(
            "TRN2", target_bir_lowering=False, debug=False, num_devices=N_CORES
        )
        _emit(nc)
        nc.compile()
        _NC_CACHE[key] = nc
    return _NC_CACHE[key]


def prep_in_maps(x, Wq, Wk, Wv, Wo):
    """Shard + cast + layout the full inputs into 8 per-core input maps."""
    x = np.asarray(x)
    Wq, Wk, Wv, Wo = (np.asarray(a) for a in (Wq, Wk, Wv, Wo))
    in_maps = []
    wqk_s = [
        np.ascontiguousarray(np.hstack([
            Wq[:, s * QCOLS:(s + 1) * QCOLS],
            Wk[:, s * KCOLS:(s + 1) * KCOLS],
        ])).astype(F16NP)
        for s in range(TP)
    ]
    wv_s = [np.ascontiguousarray(Wv[:, s * KCOLS:(s + 1) * KCOLS]).astype(F16NP)
            for s in range(TP)]
    wo_s = [np.ascontiguousarray(Wo[s * QCOLS:(s + 1) * QCOLS, :]).astype(F16NP)
            for s in range(TP)]
    xT_b = [np.ascontiguousarray(x[b].T).astype(F16NP) for b in range(B)]
    for core in range(N_CORES):
        b, s = divmod(core, TP)
        in_maps.append({
            "xT": xT_b[b],
            "wqk": wqk_s[s],
            "wv": wv_s[s],
            "wo": wo_s[s],
        })
    return in_maps


def kernel(x, Wq, Wk, Wv, Wo):
    nc = build()
    in_maps = prep_in_maps(x, Wq, Wk, Wv, Wo)
    res = run_bass_kernel_spmd(nc, in_maps, list(range(N_CORES)))
    out = np.zeros((B, L, D), np.float32)
    for core in range(N_CORES):
        b, _s = divmod(core, TP)
        out[b] += res.results[core]["out"].astype(np.float32)
    return out
